# revision 36
# baseline (speedup 1.0000x reference)
"""Trainium2 Bass kernel for nn_CopyMechanism.

Math (per batch b, one NeuronCore per batch):
  out[g,c] = softmax_c(mask ? (score_h[g]+score_c[c]) : -inf)
             * sigmoid(gate_h[g]+gate_c[c]+b0)

softmax_c of (score_h[g]+score_c[c]) == softmax_c(score_c) (score_h constant
along c), so copy_probs is g-independent and w_attn[:H] drops out.
encoder_output is unused by the reference. Scores are O(1), so exp needs no
max subtraction; masking is additive (sc - 1e5 -> sigmoid-ratio exp gives
exactly 0).

Structure (all engines pipelined under the ctx DMA stream):
  ctx streams in as 8 chunks of [128,4,1024], cast f32->bf16 in the SWDGE
  DMA (bf16 PE path: fp32 matmul runs LOW_HIGH at ~4x the cycles). Chunk
  DMAs are issued up-front so the gpsimd FIFO never stalls long. Per chunk:
  32 bf16 PE transposes -> bf16 PSUM, DVE copies to SBUF, 8 bf16 dot
  matmuls (weight pair [h,2] stationary) -> dots [2,512] f32 (gc row 0,
  sc row 1); gc broadcast (GPSIMD) + 4 sigmoids with per-partition bias gh
  (scalar) -> sig tiles; sc DMA-hopped to partition 0, additive mask, e
  via sigmoid ratio e^x = sig(x)/sig(-x) (exactly 0 when masked), partial-Z
  reduce, one e broadcast; q[gi] = sig * e_b (DVE). Post-Z tail: Z reduce,
  1/Z, per-gi q *= rZ (DVE) and 4x [128,4096] 2MB contiguous DMAs out.
"""
import sys

if "/opt/trn_rl_repo" not in sys.path:
    sys.path.insert(0, "/opt/trn_rl_repo")

import numpy as np
from contextlib import ExitStack

B, G, C, H = 8, 512, 4096, 1024
N_CORES = 8
P = 128
NCT = C // P          # 32 c-tiles of 128
NGT = G // P          # 4 g-tiles of 128
CJ = C // 512         # 8 c-chunks of 512
JH = H // P           # 8 h-blocks of 128

_cache = {}


def _build():
    import concourse.bass as bass
    import concourse.tile as tile
    from concourse import bacc, mybir
    from concourse.masks import make_identity

    f32 = mybir.dt.float32
    bf16 = mybir.dt.bfloat16
    ts = bass.ts

    nc = bacc.Bacc("TRN2", target_bir_lowering=False, debug=False,
                   num_devices=N_CORES)
    hid = nc.dram_tensor("hid", [G, H], f32, kind="ExternalInput").ap()
    ctx_d = nc.dram_tensor("ctx", [C, H], f32, kind="ExternalInput").ap()
    # additive mask row: 0.0 where mask==1 else -1e5
    madd_d = nc.dram_tensor("madd", [1, C], f32, kind="ExternalInput").ap()
    w_d = nc.dram_tensor("w", [3, H], f32, kind="ExternalInput").ap()  # wg_c, wa_c, wg_h
    bg_d = nc.dram_tensor("bg", [1, 1], f32, kind="ExternalInput").ap()
    out_d = nc.dram_tensor("out", [G, C], f32, kind="ExternalOutput").ap()

    with tile.TileContext(nc) as tc:
        with ExitStack() as ctx:
            singles = ctx.enter_context(tc.tile_pool(name="singles", bufs=1))
            hidp = ctx.enter_context(tc.tile_pool(name="hidp", bufs=1))
            ctxp = ctx.enter_context(tc.tile_pool(name="ctxp", bufs=5))
            ctp = ctx.enter_context(tc.tile_pool(name="ctp", bufs=2))
            junkp = ctx.enter_context(tc.tile_pool(name="junkp", bufs=1))
            smp = ctx.enter_context(tc.tile_pool(name="smp", bufs=1))
            qp = ctx.enter_context(tc.tile_pool(name="qp", bufs=1))
            bp = ctx.enter_context(tc.tile_pool(name="bp", bufs=2))
            sigp = ctx.enter_context(tc.tile_pool(name="sigp", bufs=2))
            rowp = ctx.enter_context(tc.tile_pool(name="rowp", bufs=2))
            tp_ps = ctx.enter_context(
                tc.tile_pool(name="tp_ps", bufs=2, space="PSUM"))
            dt_ps = ctx.enter_context(
                tc.tile_pool(name="dt_ps", bufs=2, space="PSUM"))
            z_ps_p = ctx.enter_context(
                tc.tile_pool(name="z_ps_p", bufs=2, space="PSUM"))

            # ---- ctx chunk DMAs first (SWDGE f32->bf16 cast) ----
            ctx4s = []

            def emit_ctx_dma(j, nsub=1):
                ctx4 = ctxp.tile([P, 4, H], bf16, tag="ctx4")
                w = 4 // nsub
                for h2 in range(nsub):
                    nc.gpsimd.dma_start(
                        out=ctx4[:, h2 * w:(h2 + 1) * w, :],
                        in_=ctx_d[j * 512 + h2 * w * P:
                                  j * 512 + (h2 + 1) * w * P, :].rearrange(
                            "(i p) h -> p i h", p=P))
                ctx4s.append(ctx4)

            emit_ctx_dma(0, nsub=2)
            emit_ctx_dma(1, nsub=2)

            # small inputs next on the gpsimd queue
            wpair = singles.tile([2, H], f32)
            nc.gpsimd.dma_start(out=wpair, in_=w_d[0:2, :])
            ident_b = singles.tile([P, P], bf16)
            make_identity(nc, ident_b)
            ident_f = singles.tile([2, 2], f32)
            make_identity(nc, ident_f)

            for j in range(2, 5):
                emit_ctx_dma(j)

            # hid on the HWDGE (sync) queue in f32
            hid4 = hidp.tile([P, NGT, H], f32)
            nc.sync.dma_start(out=hid4,
                              in_=hid.rearrange("(gi p) h -> p gi h", p=P))

            # ---- small constants ----
            whb = singles.tile([P, H], f32)  # wg_h broadcast to partitions
            w_gh = w_d[2:3, :]
            nc.gpsimd.dma_start(
                out=whb,
                in_=bass.AP(tensor=w_gh.tensor, offset=w_gh.offset,
                            ap=[[0, P], [1, H]]))
            bg_b = singles.tile([P, 1], f32)
            nc.gpsimd.dma_start(
                out=bg_b,
                in_=bass.AP(tensor=bg_d.tensor, offset=bg_d.offset,
                            ap=[[0, P], [1, 1]]))

            # w2[h, 2*jh + s] = w[s, jh*128 + h] for s in {0: wg_c, 1: wa_c}
            # (gc lands on dots partition 0 so GPSIMD can broadcast directly)
            w2_ps = z_ps_p.tile([P, 2 * JH], f32, tag="zps")
            for jh in range(JH):
                nc.tensor.transpose(w2_ps[:, jh * 2:jh * 2 + 2],
                                    wpair[:, ts(jh, P)], ident_f)
            w2 = singles.tile([P, 2 * JH], bf16)
            nc.scalar.copy(w2, w2_ps)

            # ---- gh = hid @ wg_h + b_gate  (column layout [128, NGT]) ----
            ghp = smp.tile([P, NGT], f32)
            for gi in range(NGT):
                junk = junkp.tile([P, H], f32, tag="junk")
                nc.vector.tensor_mul(junk, hid4[:, gi, :], whb)
                nc.vector.reduce_sum(ghp[:, gi:gi + 1], junk,
                                     axis=mybir.AxisListType.X)
            gh = smp.tile([P, NGT], f32)
            nc.vector.tensor_scalar(out=gh, in0=ghp, scalar1=bg_b[:, 0:1],
                                    scalar2=None, op0=mybir.AluOpType.add)

            # ---- persistent tiles ----
            z_row = smp.tile([1, CJ], f32)
            q = [qp.tile([P, C], f32, tag=f"q{gi}", name=f"q{gi}")
                 for gi in range(NGT)]

            # ---- per-chunk pipeline ----
            for j in range(CJ):
                # late chunk DMAs emitted mid-loop so their pool-recycle
                # waits don't head-block the gpsimd FIFO
                if j < 3:
                    emit_ctx_dma(j + 5)
                ctx4 = ctx4s[j]
                # 32 bf16 transposes -> 4 PSUM tiles of [P, 1024] bf16,
                # tile t holding h-blocks 2t,2t+1 x 4 c-tiles as [hh,i,128]
                ctxT = ctp.tile([P, JH, 512], bf16, tag="ctxT")
                for t in range(4):
                    tp = tp_ps.tile([P, 1024], bf16, tag="tps")
                    for hh in range(2):
                        jh = t * 2 + hh
                        for i in range(4):
                            nc.tensor.transpose(
                                tp[:, hh * 512 + i * P:hh * 512 + (i + 1) * P],
                                ctx4[:, i, ts(jh, P)], ident_b)
                    nc.vector.tensor_copy(
                        ctxT[:, t * 2:(t + 1) * 2, :].rearrange(
                            "p a b -> p (a b)"),
                        tp)
                dots = dt_ps.tile([2, 512], f32, tag="dots")
                for jh in range(JH):
                    nc.tensor.matmul(
                        dots, w2[:, jh * 2:jh * 2 + 2],
                        ctxT[:, jh, :],
                        start=(jh == 0), stop=(jh == JH - 1))
                scgc = rowp.tile([2, 512], f32, tag="scgc")
                nc.scalar.copy(scgc, dots)

                # gc broadcast + 4 sigmoids (bias gh[gi]) -> sig tiles
                gc_b = bp.tile([P, 512], f32, tag="gc_b")
                nc.gpsimd.partition_broadcast(gc_b, scgc[0:1, :])

                # e row: DMA-hop sc to partition 0, add mask row (from
                # DRAM), e = sig(x)/sig(-x) (exactly 0 when masked),
                # partial-Z reduce, one broadcast
                scr = rowp.tile([1, 512], f32, tag="scr")
                nc.gpsimd.dma_start(out=scr, in_=scgc[1:2, :])
                madd_j = rowp.tile([1, 512], f32, tag="madd_j")
                nc.gpsimd.dma_start(out=madd_j, in_=madd_d[0:1, ts(j, 512)])
                msc = rowp.tile([1, 512], f32, tag="msc")
                nc.vector.tensor_add(msc, scr, madd_j)
                s1 = rowp.tile([1, 512], f32, tag="s1")
                nc.scalar.activation(s1, msc,
                                     mybir.ActivationFunctionType.Sigmoid)
                s2 = rowp.tile([1, 512], f32, tag="s2")
                nc.scalar.activation(s2, msc,
                                     mybir.ActivationFunctionType.Sigmoid,
                                     scale=-1.0)
                nc.vector.reciprocal(s2, s2)
                nc.vector.tensor_mul(s1, s1, s2)
                nc.vector.reduce_sum(z_row[0:1, j:j + 1], s1,
                                     axis=mybir.AxisListType.X)
                e_b = bp.tile([P, 512], f32, tag="e_b")
                nc.gpsimd.partition_broadcast(e_b, s1)

                for gi in range(NGT):
                    sig_t = sigp.tile([P, 512], f32, tag="sig_t")
                    nc.scalar.activation(
                        sig_t, gc_b,
                        mybir.ActivationFunctionType.Sigmoid,
                        bias=gh[:, gi:gi + 1])
                    nc.vector.tensor_mul(q[gi][:, ts(j, 512)], sig_t, e_b)

            # ---- Z, 1/Z, final scale + 2MB row DMAs ----
            z1 = smp.tile([1, 1], f32)
            nc.vector.reduce_sum(z1, z_row, axis=mybir.AxisListType.X)
            rz = smp.tile([1, 1], f32)
            nc.vector.reciprocal(rz, z1)
            rz_col = smp.tile([P, 1], f32)
            nc.gpsimd.partition_broadcast(rz_col, rz)
            for gi in range(NGT):
                nc.vector.tensor_scalar(out=q[gi], in0=q[gi],
                                        scalar1=rz_col[:, 0:1],
                                        scalar2=None,
                                        op0=mybir.AluOpType.mult)
                nc.sync.dma_start(out=out_d[ts(gi, P), :], in_=q[gi])

    nc.compile()
    return nc


def _get_nc():
    if "nc" not in _cache:
        _cache["nc"] = _build()
    return _cache["nc"]


def make_w3(w_attn, w_gate):
    # rows: (wg_c, wa_c, wg_h) — gc weight first so gc lands on partition 0
    return np.ascontiguousarray(
        np.stack([w_gate[H:], w_attn[H:], w_gate[:H]], axis=0),
        dtype=np.float32)


def make_in_maps(hidden_states, context_hidden, w_attn, w_gate, b_gate,
                 copy_mask):
    w3 = make_w3(w_attn, w_gate)
    bg = np.asarray(b_gate, dtype=np.float32).reshape(1, 1)
    in_maps = []
    for b in range(B):
        madd = np.where(np.asarray(copy_mask[b]) == 0, -1e5, 0.0)
        madd = madd.reshape(1, C).astype(np.float32)
        in_maps.append({
            "hid": np.ascontiguousarray(hidden_states[b], dtype=np.float32),
            "ctx": np.ascontiguousarray(context_hidden[b], dtype=np.float32),
            "madd": np.ascontiguousarray(madd),
            "w": w3,
            "bg": bg,
        })
    return in_maps


def kernel(hidden_states, context_hidden, encoder_output, w_attn, w_gate,
           b_gate, copy_mask):
    from concourse.bass_utils import run_bass_kernel_spmd

    nc = _get_nc()
    in_maps = make_in_maps(hidden_states, context_hidden, w_attn, w_gate,
                           b_gate, copy_mask)
    res = run_bass_kernel_spmd(nc, in_maps, core_ids=list(range(N_CORES)))
    return np.stack([res.results[b]["out"] for b in range(B)], axis=0)
